# revision 2
# baseline (speedup 1.0000x reference)
"""MoE gate (DeepSeek-style group-restricted top-k router) on 8 Trainium2 cores.

Sharding: data-parallel over tokens (16384 -> 8 x 2048), gate weight
replicated; the aux-loss partial sums are combined on the host (the
all-reduce of the tiny [256] vectors).

Matmul precision: the PE's native fp32 matmul is 4 cycles/row and fp32r is
only ~2^-11 accurate, so the logits matmul runs as a 3-term fp16 split
(X=Xh+Xl, W=Wh+Wl in fp16; Xh*Wh + Xh*Wl + Xl*Wh at 1 cycle/row with fp32
PSUM accumulation). The dropped Xl*Wl term is ~2^-22 relative, giving
slightly *better* accuracy than a native fp32 matmul while running 4/3x
faster, so the top-k indices match an fp32 reference exactly.
"""
import os
import sys
from contextlib import ExitStack

import numpy as np

sys.path.insert(0, "/opt/trn_rl_repo")

import concourse.bass as bass  # noqa: E402
import concourse.tile as tile  # noqa: E402
from concourse import bacc, mybir  # noqa: E402

F32 = mybir.dt.float32
F16 = mybir.dt.float16
U32 = mybir.dt.uint32
I32 = mybir.dt.int32

N_CORES = 8
N_TOKENS = 16384
HID = 2048
NE = 256  # experts
N_GROUP = 8
GROUP = NE // N_GROUP  # 32
TOPK = 8

TOK_PC = N_TOKENS // N_CORES  # 2048 tokens per core
CHUNK = 512                   # tokens per matmul chunk
N_CHUNK = TOK_PC // CHUNK     # 4
NTILE = CHUNK // 128          # 4 token-tiles per chunk
KC = HID // 128               # 16 contraction chunks

Exp = mybir.ActivationFunctionType.Exp
Copy = mybir.ActivationFunctionType.Copy


def build_program(repeat: int = 1, mm_mode: str = "fp16x3"):
    """Build the per-core SPMD Bass program.

    repeat > 1 wraps the whole body in a hardware loop for timing runs
    (outputs are still valid; accumulators are re-zeroed each iteration).
    """
    nc = bacc.Bacc("TRN2", target_bir_lowering=False, debug=False)

    xh_d = nc.dram_tensor("xh", [HID, TOK_PC], F16, kind="ExternalInput").ap()
    xl_d = nc.dram_tensor("xl", [HID, TOK_PC], F16, kind="ExternalInput").ap()
    wh_d = nc.dram_tensor("wh", [HID, NE], F16, kind="ExternalInput").ap()
    wl_d = nc.dram_tensor("wl", [HID, NE], F16, kind="ExternalInput").ap()
    id_d = nc.dram_tensor("ident", [128, 128], F32, kind="ExternalInput").ap()
    xf_d = wf_d = None
    if mm_mode == "fp32":
        xf_d = nc.dram_tensor("xf", [HID, TOK_PC], F32, kind="ExternalInput").ap()
        wf_d = nc.dram_tensor("wf", [HID, NE], F32, kind="ExternalInput").ap()

    idx_d = nc.dram_tensor("idx", [TOK_PC, TOPK], I32, kind="ExternalOutput").ap()
    wts_d = nc.dram_tensor("wts", [TOK_PC, TOPK], F32, kind="ExternalOutput").ap()
    probs_d = nc.dram_tensor("probs", [1, NE], F32, kind="ExternalOutput").ap()

    with tile.TileContext(nc) as tc, ExitStack() as ctx:
        wpool = ctx.enter_context(tc.tile_pool(name="w", bufs=1))
        xpool = ctx.enter_context(tc.tile_pool(name="x", bufs=2))
        spool = ctx.enter_context(tc.tile_pool(name="s", bufs=2))
        tpool = ctx.enter_context(tc.tile_pool(name="t", bufs=3))
        apool = ctx.enter_context(tc.tile_pool(name="a", bufs=1))
        ppoolT = ctx.enter_context(tc.tile_pool(name="psT", bufs=2, space="PSUM"))
        ppoolL = ctx.enter_context(tc.tile_pool(name="psL", bufs=2, space="PSUM"))
        ppoolA = ctx.enter_context(tc.tile_pool(name="psA", bufs=1, space="PSUM"))

        # ---- resident weights / constants ----
        if mm_mode == "fp16x3":
            wh_sb = wpool.tile([128, KC, NE], F16)
            wl_sb = wpool.tile([128, KC, NE], F16)
            for k in range(KC):
                nc.sync.dma_start(out=wh_sb[:, k], in_=wh_d[k * 128:(k + 1) * 128])
                nc.sync.dma_start(out=wl_sb[:, k], in_=wl_d[k * 128:(k + 1) * 128])
        else:
            wf_sb = wpool.tile([128, KC, NE], F32)
            for k in range(KC):
                nc.sync.dma_start(out=wf_sb[:, k], in_=wf_d[k * 128:(k + 1) * 128])
        ones = wpool.tile([128, 1], F32)
        nc.vector.memset(ones[:], 1.0)
        ident = wpool.tile([128, 128], F32)
        nc.sync.dma_start(out=ident[:], in_=id_d[:])

        def body(_iv=None):
            # probs accumulator [128, NE], summed over token-tiles
            pacc = apool.tile([128, NE], F32)
            nc.vector.memset(pacc[:], 0.0)

            for c in range(N_CHUNK):
                # ---- load hidden chunk (transposed planes) ----
                if mm_mode == "fp16x3":
                    xh_sb = xpool.tile([128, KC, CHUNK], F16, tag="xh")
                    xl_sb = xpool.tile([128, KC, CHUNK], F16, tag="xl")
                    for k in range(KC):
                        nc.sync.dma_start(
                            out=xh_sb[:, k],
                            in_=xh_d[k * 128:(k + 1) * 128,
                                     c * CHUNK:(c + 1) * CHUNK])
                        nc.sync.dma_start(
                            out=xl_sb[:, k],
                            in_=xl_d[k * 128:(k + 1) * 128,
                                     c * CHUNK:(c + 1) * CHUNK])
                else:
                    xf_sb = xpool.tile([128, KC, CHUNK], F32, tag="xh")
                    for k in range(KC):
                        nc.sync.dma_start(
                            out=xf_sb[:, k],
                            in_=xf_d[k * 128:(k + 1) * 128,
                                     c * CHUNK:(c + 1) * CHUNK])

                # ---- logitsT = W @ X : [2x128 experts, CHUNK tokens] ----
                ltT_sb = spool.tile([128, 2, CHUNK], F32, tag="ltT")
                for e in range(2):
                    psT = ppoolT.tile([128, CHUNK], F32, tag="psT")
                    if mm_mode == "fp16x3":
                        for k in range(KC):
                            nc.tensor.matmul(
                                psT[:], lhsT=wh_sb[:, k, e * 128:(e + 1) * 128],
                                rhs=xh_sb[:, k], start=(k == 0), stop=False)
                            nc.tensor.matmul(
                                psT[:], lhsT=wh_sb[:, k, e * 128:(e + 1) * 128],
                                rhs=xl_sb[:, k], start=False, stop=False)
                            nc.tensor.matmul(
                                psT[:], lhsT=wl_sb[:, k, e * 128:(e + 1) * 128],
                                rhs=xh_sb[:, k], start=False, stop=(k == KC - 1))
                    else:
                        for k in range(KC):
                            nc.tensor.matmul(
                                psT[:], lhsT=wf_sb[:, k, e * 128:(e + 1) * 128],
                                rhs=xf_sb[:, k], start=(k == 0),
                                stop=(k == KC - 1))
                    nc.vector.tensor_copy(ltT_sb[:, e], psT[:])

                # ---- per 128-token tile: transpose + topk + softmax ----
                for tt in range(NTILE):
                    t0 = c * CHUNK + tt * 128  # global token offset in core

                    lg_ps = ppoolL.tile([128, NE], F32, tag="lg")
                    for e in range(2):
                        nc.tensor.transpose(
                            lg_ps[:, e * 128:(e + 1) * 128],
                            ltT_sb[:, e, tt * 128:(tt + 1) * 128], ident[:])
                    L = tpool.tile([128, NE], F32, tag="L")
                    nc.scalar.activation(L[:], lg_ps[:], Copy)

                    # group top-8; pooled = first 4 of each group
                    g8 = tpool.tile([128, N_GROUP, 8], F32, tag="g8")
                    for g in range(N_GROUP):
                        nc.vector.max(g8[:, g], L[:, g * GROUP:(g + 1) * GROUP])
                    top8 = tpool.tile([128, TOPK], F32, tag="top8")
                    nc.vector.max(top8[:], g8[:, :, 0:4])
                    idx_u = tpool.tile([128, TOPK], U32, tag="idxu")
                    nc.vector.max_index(idx_u[:], top8[:], L[:])
                    idx_i = tpool.tile([128, TOPK], I32, tag="idxi")
                    nc.vector.tensor_copy(idx_i[:], idx_u[:])
                    nc.sync.dma_start(out=idx_d[t0:t0 + 128], in_=idx_i[:])

                    # normalized weights
                    s8 = tpool.tile([128, 1], F32, tag="s8")
                    nc.vector.reduce_sum(s8[:], top8[:], axis=mybir.AxisListType.X)
                    nc.vector.tensor_scalar_add(s8[:], s8[:], 1e-20)
                    r8 = tpool.tile([128, 1], F32, tag="r8")
                    nc.vector.reciprocal(r8[:], s8[:])
                    w8 = tpool.tile([128, TOPK], F32, tag="w8")
                    nc.vector.tensor_scalar_mul(w8[:], top8[:], r8[:])
                    nc.sync.dma_start(out=wts_d[t0:t0 + 128], in_=w8[:])

                    # softmax probs, accumulated over tiles
                    m = tpool.tile([128, 1], F32, tag="m")
                    nc.vector.reduce_max(m[:], L[:], axis=mybir.AxisListType.X)
                    nm = tpool.tile([128, 1], F32, tag="nm")
                    nc.vector.tensor_scalar_mul(nm[:], m[:], -1.0)
                    ex = tpool.tile([128, NE], F32, tag="ex")
                    ssum = tpool.tile([128, 1], F32, tag="ssum")
                    nc.scalar.activation(ex[:], L[:], Exp, bias=nm[:],
                                         scale=1.0, accum_out=ssum[:])
                    rs = tpool.tile([128, 1], F32, tag="rs")
                    nc.vector.reciprocal(rs[:], ssum[:])
                    # pacc += ex * rs
                    nc.vector.scalar_tensor_tensor(
                        out=pacc[:], in0=ex[:], scalar=rs[:], in1=pacc[:],
                        op0=mybir.AluOpType.mult, op1=mybir.AluOpType.add)

            # ---- partition-reduce probs: [1, NE] ----
            ps_p = ppoolA.tile([1, NE], F32)
            nc.tensor.matmul(ps_p[:], lhsT=ones[:], rhs=pacc[:],
                             start=True, stop=True)
            probs_sb = tpool.tile([1, NE], F32, tag="pr")
            nc.vector.tensor_copy(probs_sb[:], ps_p[:])
            nc.sync.dma_start(out=probs_d[:], in_=probs_sb[:])

        if repeat > 1:
            with tc.For_i(0, repeat, 1) as iv:
                body(iv)
        else:
            body()

    nc.compile()
    return nc


_CACHE = {}


def _get_program(repeat: int, mm_mode: str):
    key = (repeat, mm_mode)
    if key not in _CACHE:
        _CACHE[key] = build_program(repeat, mm_mode)
    return _CACHE[key]


def _host_prep(hidden_states: np.ndarray, weight: np.ndarray, mm_mode: str):
    """Shard + transpose + fp16-split on the host."""
    hs = np.asarray(hidden_states, dtype=np.float32)
    w = np.asarray(weight, dtype=np.float32)
    wT = np.ascontiguousarray(w.T)  # [HID, NE]
    ident = np.eye(128, dtype=np.float32)
    in_maps = []
    if mm_mode == "fp16x3":
        wh = wT.astype(np.float16)
        wl = (wT - wh.astype(np.float32)).astype(np.float16)
        for c in range(N_CORES):
            shard = hs[c * TOK_PC:(c + 1) * TOK_PC]  # [TOK_PC, HID]
            xT = np.ascontiguousarray(shard.T)       # [HID, TOK_PC]
            xh = xT.astype(np.float16)
            xl = (xT - xh.astype(np.float32)).astype(np.float16)
            in_maps.append({"xh": xh, "xl": xl, "wh": wh, "wl": wl,
                            "ident": ident})
    else:
        z16 = np.zeros((HID, TOK_PC), np.float16)
        zw16 = np.zeros((HID, NE), np.float16)
        for c in range(N_CORES):
            shard = hs[c * TOK_PC:(c + 1) * TOK_PC]
            xT = np.ascontiguousarray(shard.T)
            in_maps.append({"xf": xT, "wf": wT, "ident": ident})
    return in_maps


def run(hidden_states, weight, repeat: int = 1, mm_mode: str | None = None):
    from concourse.bass_utils import run_bass_kernel_spmd

    mm_mode = mm_mode or os.environ.get("MOE_MM_MODE", "fp16x3")
    nc = _get_program(repeat, mm_mode)
    in_maps = _host_prep(hidden_states, weight, mm_mode)
    res = run_bass_kernel_spmd(nc, in_maps, list(range(N_CORES)))

    idx = np.concatenate([res.results[c]["idx"] for c in range(N_CORES)], axis=0)
    wts = np.concatenate([res.results[c]["wts"] for c in range(N_CORES)], axis=0)
    probs_sum = np.sum([res.results[c]["probs"][0].astype(np.float64)
                        for c in range(N_CORES)], axis=0)
    # host all-reduce of the aux-loss pieces
    counts = np.bincount(idx.ravel(), minlength=NE).astype(np.float64)
    counts /= float(N_TOKENS * TOPK)
    probs = probs_sum / float(N_TOKENS)
    aux = np.float32(0.001 * np.dot(counts, probs))
    return idx.astype(np.int32), wts.astype(np.float32), aux


def kernel(hidden_states, weight):
    idx, wts, aux = run(hidden_states, weight)
    return idx, wts, aux
